# revision 4
# baseline (speedup 1.0000x reference)
"""Multi-head attention (B=2, S=4096, D=1024, H=16) on 8 NeuronCores.

Sharding: core c = (batch b = c // 4, head-group g = c % 4).  Each head-group
owns 4 heads = 256 projection features.  All device compute in bf16
(fp32 PSUM accumulation); host pre-transposes and casts inputs so the
kernel does zero on-chip transposition of activations or weights:
  - host supplies qT/kT/vT [D, S] bf16, wqT/wkT/wvT [D, E] bf16,
    w0T [E, D] bf16 per core
  - projections contract over d in 8 chunks of 128 (fp32 PSUM)
  - attention per head with scores transposed ([k, q]); softmax
    normalization deferred via a ones column in the PV stationary operand
    (row 64 of the PV output = exp row-sums); exp computes
    exp(score/8 - 2) on ACT -- the -2 bias cancels in the softmax ratio
  - per-head normalization (transpose -> scale by 1/sum -> transpose back)
  - output projection -> partial [S, D] bf16; host sums the 4 partials
    per batch in fp32.
"""

import numpy as np
from contextlib import ExitStack

import concourse.bass as bass
import concourse.bacc as bacc
import concourse.tile as tile
from concourse import mybir, bass_utils
from concourse.masks import make_identity
import ml_dtypes

B, S, D, H = 2, 4096, 1024, 16
DK = D // H          # 64
NCORES = 8
GROUPS = 4           # head-groups (tensor parallel)
HG = H // GROUPS     # 4 heads per group
E = HG * DK          # 256 features per group

F32 = mybir.dt.float32
BF16 = mybir.dt.bfloat16
NPBF16 = ml_dtypes.bfloat16

P = 128              # partitions
DC = D // P          # 8 d-chunks
SC = S // P          # 32 s-chunks of 128
SW = 1024            # projection staging window along s
NW = S // SW         # 4
QB = 1024            # q-block in attention
NQB = S // QB        # 4
NST = SC             # 32 k-stripes of 128
VW = DK + 1          # vp columns per head incl. ones column (65)
VPAD = 66            # padded per-head stride in vps tile
EXP_BIAS = -2.0      # exp(s/8 - 2): keeps exp outputs small; cancels in ratio


def kernel_body(tc, qT, kT, vT, wqT, wkT, wvT, w0T, out):
    nc = tc.nc
    ctx = ExitStack()
    with ctx:
        ident_pool = ctx.enter_context(tc.tile_pool(name="ident", bufs=1))
        identity = ident_pool.tile([P, P], F32)
        make_identity(nc, identity)
        ebias = ident_pool.tile([P, 1], F32, tag="ebias", name="ebias")
        nc.vector.memset(ebias, EXP_BIAS)

        # persistent across A..W
        w0s_pool = ctx.enter_context(tc.tile_pool(name="w0s", bufs=1))
        w0s = w0s_pool.tile([P, 2, D], BF16, tag="w0s", name="w0s")
        for ec in range(2):
            nc.sync.dma_start(out=w0s[:, ec, :], in_=w0T[ec * P:(ec + 1) * P, :])

        # persistent through phase A
        proj_ctx = ExitStack()
        proj_pool = proj_ctx.enter_context(tc.tile_pool(name="proj", bufs=1))
        qpT = [proj_pool.tile([P, S], BF16, tag=f"qpT{i}", name=f"qpT{i}")
               for i in range(2)]
        kpT = [proj_pool.tile([P, S], BF16, tag=f"kpT{i}", name=f"kpT{i}")
               for i in range(2)]
        vps = proj_pool.tile([P, SC, HG * VPAD], BF16, tag="vps", name="vps")

        # ================= phase T: load + projections =================
        with tc.tile_pool(name="t_w", bufs=1) as wpool, \
             tc.tile_pool(name="t_x", bufs=2) as xpool, \
             tc.tile_pool(name="t_ps", bufs=4, space="PSUM") as pspool:
            wqs = wpool.tile([P, DC, E], BF16, tag="wqs", name="wqs")
            wks = wpool.tile([P, DC, E], BF16, tag="wks", name="wks")
            wvs = wpool.tile([P, DC, E], BF16, tag="wvs", name="wvs")
            for wsrc, wdst in ((wqT, wqs), (wkT, wks), (wvT, wvs)):
                for dc in range(DC):
                    nc.sync.dma_start(out=wdst[:, dc, :],
                                      in_=wsrc[dc * P:(dc + 1) * P, :])

            for w in range(NW):
                s0 = w * SW
                for kind, src in ((0, qT), (1, kT), (2, vT)):
                    xst = xpool.tile([P, DC, SW], BF16, tag="xst", name="xst")
                    for dc in range(DC):
                        nc.sync.dma_start(
                            out=xst[:, dc, :],
                            in_=src[dc * P:(dc + 1) * P, s0:s0 + SW])
                    if kind < 2:
                        wT = wqs if kind == 0 else wks
                        dst = qpT if kind == 0 else kpT
                        for et in range(2):
                            for sb in range(SW // 512):
                                acc = pspool.tile([P, 512], F32, tag="acc",
                                                  name="acc")
                                for dc in range(DC):
                                    nc.tensor.matmul(
                                        acc,
                                        wT[:, dc, et * P:(et + 1) * P],
                                        xst[:, dc, sb * 512:(sb + 1) * 512],
                                        start=(dc == 0), stop=(dc == DC - 1))
                                nc.vector.tensor_copy(
                                    out=dst[et][:, s0 + sb * 512:
                                                s0 + (sb + 1) * 512],
                                    in_=acc)
                    else:
                        for sc4 in range(SW // P):
                            scg = w * (SW // P) + sc4
                            accv = pspool.tile([P, E], F32, tag="accv",
                                               name="accv")
                            for dc in range(DC):
                                nc.tensor.matmul(
                                    accv,
                                    xst[:, dc, sc4 * P:(sc4 + 1) * P],
                                    wvs[:, dc, :],
                                    start=(dc == 0), stop=(dc == DC - 1))
                            nc.vector.tensor_copy(
                                out=vps[:, scg, :].rearrange(
                                    "p (h c) -> p h c", c=VPAD)[:, :, 0:DK],
                                in_=accv.rearrange("p (h c) -> p h c", c=DK))
            # ones column for the PV sums row
            ones_sc = wpool.tile([P, SC], F32, tag="ones_sc", name="ones_sc")
            nc.vector.memset(ones_sc, 1.0)
            for h in range(HG):
                nc.vector.tensor_copy(
                    out=vps[:, :, h * VPAD + DK:h * VPAD + DK + 1],
                    in_=ones_sc.rearrange("p (s o) -> p s o", o=1))

        # ================= phase A: attention =================
        x65_pool = ctx.enter_context(
            tc.tile_pool(name="x65", bufs=1, side="right"))
        x65 = [x65_pool.tile([VW, S], F32, tag=f"x65_{h}", name=f"x65_{h}")
               for h in range(HG)]
        with tc.tile_pool(name="a_att", bufs=3) as att_pool, \
             tc.tile_pool(name="a_st", bufs=2, space="PSUM") as ppool_st, \
             tc.tile_pool(name="a_x", bufs=2, space="PSUM") as ppool_x:
            for h in range(HG):
                et, hp = h // 2, (h % 2) * DK
                for qb in range(NQB):
                    q0 = qb * QB
                    xacc = ppool_x.tile([VW, QB], F32, tag="xacc", name="xacc")
                    for kk in range(NST):
                        st = ppool_st.tile([P, QB], F32, tag="st", name="st")
                        lhs_k = kpT[et][hp:hp + DK, kk * P:(kk + 1) * P]
                        for j in range(QB // 512):
                            nc.tensor.matmul(
                                st[:, j * 512:(j + 1) * 512],
                                lhs_k,
                                qpT[et][hp:hp + DK,
                                        q0 + j * 512:q0 + (j + 1) * 512],
                                start=True, stop=True)
                        attst = att_pool.tile([P, QB], BF16, tag="att",
                                              name="att")
                        nc.scalar.activation(
                            attst, st, mybir.ActivationFunctionType.Exp,
                            bias=ebias, scale=0.125)
                        lhs_v = vps[:, kk, h * VPAD:h * VPAD + VW]
                        for j in range(QB // 512):
                            nc.tensor.matmul(
                                xacc[:, j * 512:(j + 1) * 512],
                                lhs_v,
                                attst[:, j * 512:(j + 1) * 512],
                                start=(kk == 0), stop=(kk == NST - 1))
                    nc.vector.tensor_copy(out=x65[h][:, q0:q0 + QB], in_=xacc)
        proj_ctx.close()   # release qpT/kpT/vps

        # ============ phase N: normalize x by 1/rowsum ============
        xw_pool = ctx.enter_context(
            tc.tile_pool(name="xw", bufs=1, side="right"))
        xw = [xw_pool.tile([P, S], BF16, tag=f"xw{i}", name=f"xw{i}")
              for i in range(2)]
        with tc.tile_pool(name="n_sb", bufs=4) as nsb_pool, \
             tc.tile_pool(name="n_pt", bufs=3, space="PSUM") as ppool_nt, \
             tc.tile_pool(name="n_pb", bufs=2, space="PSUM") as ppool_nb:
            for et in range(2):
                for qc in range(SC):
                    xs2 = nsb_pool.tile([P, 2 * DK], F32, tag="xs2",
                                        name="xs2")
                    for hp2 in range(2):
                        h = 2 * et + hp2
                        tp = ppool_nt.tile([P, VW], F32, tag="ntp", name="ntp")
                        nc.tensor.transpose(
                            tp, x65[h][:, qc * P:(qc + 1) * P],
                            identity[:VW, :VW])
                        rcp = nsb_pool.tile([P, 1], F32, tag="rcp", name="rcp")
                        nc.vector.reciprocal(rcp, tp[:, DK:DK + 1])
                        nc.vector.tensor_scalar_mul(
                            xs2[:, hp2 * DK:(hp2 + 1) * DK],
                            tp[:, 0:DK], rcp)
                    tb = ppool_nb.tile([P, P], F32, tag="ntb", name="ntb")
                    nc.tensor.transpose(tb, xs2, identity)
                    nc.vector.tensor_copy(
                        out=xw[et][:, qc * P:(qc + 1) * P], in_=tb)

        # ================= phase W: output projection =================
        with tc.tile_pool(name="w_sb", bufs=3) as osb_pool, \
             tc.tile_pool(name="w_ps", bufs=3, space="PSUM") as ppool_w:
            for qc in range(SC):
                oacc = ppool_w.tile([P, D], F32, tag="oacc", name="oacc")
                for ec in range(2):
                    for j in range(2):
                        nc.tensor.matmul(
                            oacc[:, j * 512:(j + 1) * 512],
                            xw[ec][:, qc * P:(qc + 1) * P],
                            w0s[:, ec, j * 512:(j + 1) * 512],
                            start=(ec == 0), stop=(ec == 1))
                osb = osb_pool.tile([P, D], BF16, tag="osb", name="osb")
                nc.vector.tensor_copy(out=osb, in_=oacc)
                nc.sync.dma_start(out=out[qc * P:(qc + 1) * P, :], in_=osb)


def build_program():
    nc = bacc.Bacc("TRN2", target_bir_lowering=False, debug=False,
                   num_devices=NCORES)
    qT = nc.dram_tensor("qT", (D, S), BF16, kind="ExternalInput").ap()
    kT = nc.dram_tensor("kT", (D, S), BF16, kind="ExternalInput").ap()
    vT = nc.dram_tensor("vT", (D, S), BF16, kind="ExternalInput").ap()
    wqT = nc.dram_tensor("wqT", (D, E), BF16, kind="ExternalInput").ap()
    wkT = nc.dram_tensor("wkT", (D, E), BF16, kind="ExternalInput").ap()
    wvT = nc.dram_tensor("wvT", (D, E), BF16, kind="ExternalInput").ap()
    w0T = nc.dram_tensor("w0T", (E, D), BF16, kind="ExternalInput").ap()
    out = nc.dram_tensor("out", (S, D), BF16, kind="ExternalOutput").ap()
    with tile.TileContext(nc) as tc:
        kernel_body(tc, qT, kT, vT, wqT, wkT, wvT, w0T, out)
    nc.compile()
    return nc


_NC_CACHE = None


def _get_program():
    global _NC_CACHE
    if _NC_CACHE is None:
        _NC_CACHE = build_program()
    return _NC_CACHE


def make_in_maps(q, k, v, wq, wk, wv, w0):
    arrs = [np.asarray(a, dtype=np.float32)
            for a in (q, k, v, wq, wk, wv, w0)]
    q, k, v, wq, wk, wv, w0 = arrs
    qTb = [np.ascontiguousarray(q[b].astype(NPBF16).T) for b in range(B)]
    kTb = [np.ascontiguousarray(k[b].astype(NPBF16).T) for b in range(B)]
    vTb = [np.ascontiguousarray(v[b].astype(NPBF16).T) for b in range(B)]
    in_maps = []
    for c in range(NCORES):
        b, g = c // GROUPS, c % GROUPS
        e0 = g * E
        in_maps.append({
            "qT": qTb[b],
            "kT": kTb[b],
            "vT": vTb[b],
            "wqT": np.ascontiguousarray(wq[e0:e0 + E, :].astype(NPBF16).T),
            "wkT": np.ascontiguousarray(wk[e0:e0 + E, :].astype(NPBF16).T),
            "wvT": np.ascontiguousarray(wv[e0:e0 + E, :].astype(NPBF16).T),
            "w0T": np.ascontiguousarray(w0[:, e0:e0 + E].astype(NPBF16).T),
        })
    return in_maps


def gather_out(results):
    out = np.zeros((B, S, D), dtype=np.float32)
    for c in range(NCORES):
        b = c // GROUPS
        out[b] += results[c]["out"].astype(np.float32)
    return out


def _install_ntff_hook_shim():
    """This image's antenv lacks axon_hooks; recreate it so trace=True works.

    Mirrors trn_agent_boot.trn_boot._ntff_profile_via_ctypes against
    /opt/axon/libaxon_pjrt.so.
    """
    import sys, types, ctypes, contextlib
    if "antenv.axon_hooks" in sys.modules:
        return
    mod = types.ModuleType("antenv.axon_hooks")
    mod._hook = None

    def set_axon_ntff_profile_hook(h):
        mod._hook = h

    def get_axon_ntff_profile_hook():
        return mod._hook

    mod.set_axon_ntff_profile_hook = set_axon_ntff_profile_hook
    mod.get_axon_ntff_profile_hook = get_axon_ntff_profile_hook
    sys.modules["antenv.axon_hooks"] = mod
    try:
        import antenv
        antenv.axon_hooks = mod
    except ImportError:
        pass

    so_path = "/opt/axon/libaxon_pjrt.so"
    try:
        lib = ctypes.CDLL(so_path)
        if not hasattr(lib, "axon_start_nrt_profile"):
            return
        lib.axon_start_nrt_profile.argtypes = [
            ctypes.POINTER(ctypes.c_int64), ctypes.c_size_t]
        lib.axon_start_nrt_profile.restype = ctypes.c_int64
        lib.axon_stop_nrt_profile.argtypes = [ctypes.c_char_p]
        lib.axon_stop_nrt_profile.restype = ctypes.c_int64
    except OSError:
        return

    @contextlib.contextmanager
    def _hook(output_dir, device_ids):
        import jax
        jax.devices()
        if device_ids:
            ids = (ctypes.c_int64 * len(device_ids))(*device_ids)
            rc = lib.axon_start_nrt_profile(ids, len(device_ids))
        else:
            rc = lib.axon_start_nrt_profile(None, 0)
        if rc != 0:
            raise RuntimeError(f"axon_start_nrt_profile rc={rc}")
        try:
            yield
        finally:
            n = lib.axon_stop_nrt_profile(str(output_dir).encode())
            print(f"profile: {n} file(s) written to {output_dir}")

    mod._hook = _hook


def kernel(q, k, v, wq, wk, wv, w0, _trace=False, _tmpdir=None):
    if _trace:
        _install_ntff_hook_shim()
    nc = _get_program()
    in_maps = make_in_maps(q, k, v, wq, wk, wv, w0)
    res = bass_utils.run_bass_kernel_spmd(
        nc, in_maps, core_ids=list(range(NCORES)),
        trace=_trace, tmpdir=_tmpdir)
    out = gather_out(res.results)
    if _trace:
        return out, res
    return out
